# revision 13
# baseline (speedup 1.0000x reference)
"""DeepSeek-MoE Trainium2 kernel (8-core expert-parallel).

Strategy (per spec sharding_hint): expert-parallel. Each of the 8 cores owns
8 of the 64 routed experts. The host computes the router (grouped top-k) and
dispatches: tokens are gathered per expert (transposed, so the contraction
dim H lands on SBUF partitions), padded to a per-slot capacity that is
uniform across cores so one Bass program serves all 8 cores SPMD. The device
streams each expert's weights from HBM exactly once and runs the SwiGLU
matmuls in float32r (full-rate PE) with fp32 accumulation, scaling each
token row by its combine weight. Shared experts are token-sharded: core m
computes the shared MLP for tokens [512m, 512(m+1)). The host scatter-adds
per-expert outputs back to token order (the unshard step).

Matmul layout trick: h1^T = matmul(lhsT=W_gate[H,I], rhs=x^T[H,C]) gives
[I, C] directly, and y = matmul(lhsT=a^T[I,C], rhs=W_down[I,H]) gives
[C, H] — no on-device transposes anywhere.
"""

import os
import numpy as np

# ---- problem constants (nn_DeepseekMoE_42236708389026) ----
T, H = 4096, 1024
E, I = 64, 704
IP = 768                    # I padded to a multiple of 128 (zero-padded weights)
TOP_K, N_GROUP, TOPK_GROUP = 8, 8, 4
N_SHARED = 2
I2 = N_SHARED * I           # 1408 = 11 * 128
ROUTED_SCALE = 2.5
NCORES = 8
EL = E // NCORES            # 8 local experts / core
TS = T // NCORES            # 512 shared-slab tokens / core
P = 128
NT = 512                    # token tile (PSUM bank = 512 fp32)
KO = H // P                 # 8

_BUILD_CACHE: dict = {}
LAST_EXEC_NS = None
LAST_RESULTS = None


def _routing(x, gate_w, gate_bias):
    """Replicates the reference _grouped_topk bit-exactly (jax on CPU)."""
    import jax
    import jax.numpy as jnp

    cpu = jax.devices("cpu")[0]
    with jax.default_device(cpu):
        x = jnp.asarray(x)
        gate_w = jnp.asarray(gate_w)
        gate_bias = jnp.asarray(gate_bias)
        logits = jnp.einsum("th,eh->te", x, gate_w)
        scores = jax.nn.sigmoid(logits)
        sc = scores + gate_bias[None, :]
        g = sc.reshape(-1, N_GROUP, E // N_GROUP)
        group_scores = jnp.sum(jax.lax.top_k(g, 2)[0], axis=-1)
        _, group_idx = jax.lax.top_k(group_scores, TOPK_GROUP)
        group_mask = jnp.sum(jax.nn.one_hot(group_idx, N_GROUP, dtype=sc.dtype), axis=1) > 0
        masked = jnp.where(group_mask[:, :, None], g, -jnp.inf).reshape(-1, E)
        _, topk_idx = jax.lax.top_k(masked, TOP_K)
        topk_w = jnp.take_along_axis(scores, topk_idx, axis=-1)
        topk_w = topk_w / jnp.sum(topk_w, axis=-1, keepdims=True)
        topk_w = topk_w * ROUTED_SCALE
        return np.asarray(topk_w, np.float32), np.asarray(topk_idx, np.int32)


def _token_tiles(C):
    out = []
    off = 0
    while off < C:
        sz = min(NT, C - off)
        out.append((off, sz))
        off += sz
    return out


def _emit_swiglu_slot(nc, tc, pools, IT, xgt, xcol0, C, wg_s, wu_s, wd_s,
                      cw_t, out, orow0, tagp):
    """One expert slot: out[orow0:orow0+C] = swiglu(x) (optionally row-scaled).

    xgt: DRAM [H, *] token matrix (transposed); columns [xcol0, xcol0+C).
    wg_s/wu_s: DRAM [H, IT*128] ; wd_s: DRAM [IT*128, H].
    cw_t: SBUF [P, ncols] per-chunk combine weights (col = global row / 128),
          or None for the shared slot.
    out: DRAM [*, H]; rows [orow0, orow0+C).
    """
    import concourse.mybir as mybir
    F32 = mybir.dt.float32
    F32R = mybir.dt.float32r
    AF = mybir.ActivationFunctionType
    wpool, wdpool, xpool, apool, ypool, pp = pools
    IF = IT * P

    wg_t = wpool.tile([P, KO, IF], F32R, tag=f"{tagp}wg")
    wu_t = wpool.tile([P, KO, IF], F32R, tag=f"{tagp}wu")
    wd_t = wdpool.tile([P, IT, H], F32R, tag=f"{tagp}wd")
    for ko in range(KO):
        nc.sync.dma_start(wg_t[:, ko], wg_s[ko * P:(ko + 1) * P, :])
        nc.sync.dma_start(wu_t[:, ko], wu_s[ko * P:(ko + 1) * P, :])
    for it in range(IT):
        nc.sync.dma_start(wd_t[:, it], wd_s[it * P:(it + 1) * P, :])

    for (ntoff, ntsz) in _token_tiles(C):
        xg_t = xpool.tile([P, KO, NT], F32R, tag=f"{tagp}xg", name="xg_t")[:, :, :ntsz]
        for ko in range(KO):
            nc.sync.dma_start(
                xg_t[:, ko],
                xgt[ko * P:(ko + 1) * P, xcol0 + ntoff: xcol0 + ntoff + ntsz])

        a_t = apool.tile([P, IT, NT], F32R, tag=f"{tagp}aT", name="a_t")[:, :, :ntsz]
        for it in range(IT):
            ps1 = pp.tile([P, NT], F32, tag="ps1", name="ps1")[:, :ntsz]
            ps2 = pp.tile([P, NT], F32, tag="ps2", name="ps2")[:, :ntsz]
            for ko in range(KO):
                nc.tensor.matmul(
                    ps1,
                    lhsT=wg_t[:, ko, it * P:(it + 1) * P],
                    rhs=xg_t[:, ko],
                    start=(ko == 0), stop=(ko == KO - 1))
                nc.tensor.matmul(
                    ps2,
                    lhsT=wu_t[:, ko, it * P:(it + 1) * P],
                    rhs=xg_t[:, ko],
                    start=(ko == 0), stop=(ko == KO - 1))
            nc.scalar.activation(a_t[:, it], ps1, AF.Sigmoid)
            nc.vector.tensor_mul(a_t[:, it], a_t[:, it], ps1)
            nc.vector.tensor_mul(a_t[:, it], a_t[:, it], ps2)

        for ch in range((ntsz + P - 1) // P):
            m = min(P, ntsz - ch * P)
            for hh in range(H // NT):
                ps3 = pp.tile([P, NT], F32, tag="ps3", name="ps3")[:m]
                for it in range(IT):
                    nc.tensor.matmul(
                        ps3,
                        lhsT=a_t[:, it, ch * P: ch * P + m],
                        rhs=wd_t[:, it, hh * NT:(hh + 1) * NT],
                        start=(it == 0), stop=(it == IT - 1))
                y_t = ypool.tile([P, NT], F32, tag=f"{tagp}y", name="y_t")[:m]
                row0 = orow0 + ntoff + ch * P
                if cw_t is not None:
                    nc.vector.tensor_scalar_mul(y_t, ps3, cw_t[:m, row0 // P, None])
                else:
                    nc.vector.tensor_copy(y_t, ps3)
                nc.sync.dma_start(out[row0: row0 + m, hh * NT:(hh + 1) * NT], y_t)


def _build(Cs, offs, CT):
    """Build + schedule the SPMD Bass program for slot capacities Cs."""
    import concourse.mybir as mybir
    from concourse import bacc
    import concourse.tile as tile

    F32 = mybir.dt.float32
    F32R = mybir.dt.float32r

    nc = bacc.Bacc("TRN2", target_bir_lowering=False, debug=False)
    xgt = nc.dram_tensor("xgt", [H, CT], F32R, kind="ExternalInput")
    cw = nc.dram_tensor("cw", [CT], F32, kind="ExternalInput")
    wg = nc.dram_tensor("wg", [EL, H, IP], F32R, kind="ExternalInput")
    wu = nc.dram_tensor("wu", [EL, H, IP], F32R, kind="ExternalInput")
    wd = nc.dram_tensor("wd", [EL, IP, H], F32R, kind="ExternalInput")
    xst = nc.dram_tensor("xst", [H, TS], F32R, kind="ExternalInput")
    wsg = nc.dram_tensor("wsg", [H, I2], F32R, kind="ExternalInput")
    wsu = nc.dram_tensor("wsu", [H, I2], F32R, kind="ExternalInput")
    wsd = nc.dram_tensor("wsd", [I2, H], F32R, kind="ExternalInput")
    y = nc.dram_tensor("y", [CT, H], F32, kind="ExternalOutput")
    ys = nc.dram_tensor("ys", [TS, H], F32, kind="ExternalOutput")

    with tile.TileContext(nc) as tc:
        with tc.tile_pool(name="psum", bufs=2, space="PSUM") as pp:
            # --- routed expert slots ---
            with (
                tc.tile_pool(name="rw", bufs=2) as wpool,
                tc.tile_pool(name="rwd", bufs=1) as wdpool,
                tc.tile_pool(name="rx", bufs=2) as xpool,
                tc.tile_pool(name="ra", bufs=2) as apool,
                tc.tile_pool(name="ry", bufs=2) as ypool,
                tc.tile_pool(name="rc", bufs=1) as cpool,
            ):
                cw_t = cpool.tile([P, CT // P], F32, tag="cw")
                nc.sync.dma_start(cw_t, cw.rearrange("(n p) -> p n", p=P))
                for s in range(EL):
                    C = int(Cs[s])
                    if C:
                        _emit_swiglu_slot(
                            nc, tc, (wpool, wdpool, xpool, apool, ypool, pp),
                            IP // P, xgt, int(offs[s]), C,
                            wg[s], wu[s], wd[s], cw_t, y, int(offs[s]), "r")
            # --- shared experts (token shard) ---
            with (
                tc.tile_pool(name="sw", bufs=1) as swpool,
                tc.tile_pool(name="sx", bufs=1) as sxpool,
                tc.tile_pool(name="sa", bufs=1) as sapool,
                tc.tile_pool(name="sy", bufs=2) as sypool,
            ):
                _emit_swiglu_slot(
                    nc, tc, (swpool, swpool, sxpool, sapool, sypool, pp),
                    I2 // P, xst, 0, TS,
                    wsg, wsu, wsd, None, ys, 0, "s")

    nc.compile()
    return nc


def _prepare(hidden_states, gate_w, gate_bias, w_gate, w_up, w_down,
             ws_gate, ws_up, ws_down):
    """Host routing + dispatch. Returns (nc, in_maps, meta) for the SPMD run."""
    hs = np.ascontiguousarray(np.asarray(hidden_states, np.float32))
    gate_w = np.asarray(gate_w, np.float32)
    gate_bias = np.asarray(gate_bias, np.float32)
    w_gate = np.asarray(w_gate, np.float32)
    w_up = np.asarray(w_up, np.float32)
    w_down = np.asarray(w_down, np.float32)
    ws_gate = np.ascontiguousarray(np.asarray(ws_gate, np.float32))
    ws_up = np.ascontiguousarray(np.asarray(ws_up, np.float32))
    ws_down = np.ascontiguousarray(np.asarray(ws_down, np.float32))

    # ---- host: router + dispatch (the sharding step) ----
    topk_w, topk_idx = _routing(hs, gate_w, gate_bias)

    rows_of = []
    wts_of = []
    counts = np.zeros(E, np.int64)
    for e in range(E):
        rr, kk = np.nonzero(topk_idx == e)
        rows_of.append(rr)
        wts_of.append(topk_w[rr, kk])
        counts[e] = len(rr)

    # expert -> (core, slot) assignment: we own the sharding, so sort experts
    # by token count desc and give slot s of core m the (8s+m)-th largest.
    # Sorted tiering minimizes sum-of-tier-maxima = padded compute per core.
    order = np.argsort(-counts, kind="stable")
    perm = order.reshape(EL, NCORES).T              # [NCORES, EL]
    slot_counts = counts[perm]                      # [NCORES, EL]
    # capacity = tier max rounded up to even (fp32r ISA: innermost moving /
    # psum-dst counts must be even — fp32r is a bf16-pair decomposition)
    Cs = ((slot_counts.max(axis=0) + 1) // 2 * 2).astype(np.int64)
    # buffer offsets 128-aligned (cw chunk addressing); capacities exact
    offs = np.concatenate([[0], np.cumsum((Cs + P - 1) // P * P)[:-1]])
    CT = int(((Cs[-1] + P - 1) // P * P) + offs[-1])

    hsT = np.ascontiguousarray(hs.T)                # [H, T]

    in_maps = []
    for m in range(NCORES):
        xgt_m = np.zeros((H, CT), np.float32)
        cw_m = np.zeros(CT, np.float32)
        wg_m = np.zeros((EL, H, IP), np.float32)
        wu_m = np.zeros((EL, H, IP), np.float32)
        wd_m = np.zeros((EL, IP, H), np.float32)
        for s in range(EL):
            e = perm[m, s]
            n = counts[e]
            o = offs[s]
            xgt_m[:, o:o + n] = hsT[:, rows_of[e]]
            cw_m[o:o + n] = wts_of[e]
            wg_m[s, :, :I] = w_gate[e]
            wu_m[s, :, :I] = w_up[e]
            wd_m[s, :I, :] = w_down[e]
        in_maps.append(dict(
            xgt=xgt_m, cw=cw_m, wg=wg_m, wu=wu_m, wd=wd_m,
            xst=np.ascontiguousarray(hsT[:, m * TS:(m + 1) * TS]),
            wsg=ws_gate, wsu=ws_up, wsd=ws_down))

    key = tuple(int(c) for c in Cs)
    if key not in _BUILD_CACHE:
        _BUILD_CACHE[key] = _build(Cs, offs, CT)
    nc = _BUILD_CACHE[key]

    meta = dict(perm=perm, counts=counts, offs=offs, rows_of=rows_of)
    return nc, in_maps, meta


def _combine(results, meta):
    """Host unshard: scatter-add per-expert outputs back to token order."""
    perm, counts, offs, rows_of = (
        meta["perm"], meta["counts"], meta["offs"], meta["rows_of"])
    out = np.zeros((T, H), np.float32)
    for m in range(NCORES):
        y_m = results[m]["y"]
        for s in range(EL):
            e = perm[m, s]
            n = counts[e]
            o = offs[s]
            out[rows_of[e]] += y_m[o:o + n]
        out[m * TS:(m + 1) * TS] += results[m]["ys"]
    return out


def kernel(hidden_states, gate_w, gate_bias, w_gate, w_up, w_down,
           ws_gate, ws_up, ws_down):
    from concourse import bass_utils

    nc, in_maps, meta = _prepare(
        hidden_states, gate_w, gate_bias, w_gate, w_up, w_down,
        ws_gate, ws_up, ws_down)
    res = bass_utils.run_bass_kernel_spmd(
        nc, in_maps, core_ids=list(range(NCORES)))
    return _combine(res.results, meta)


# revision 15
# speedup vs baseline: 1.8700x; 1.8700x over previous
"""DeepSeek-MoE Trainium2 kernel (8-core expert-parallel).

Strategy (per spec sharding_hint): expert-parallel. Each of the 8 cores owns
8 of the 64 routed experts. The host computes the router (grouped top-k) and
dispatches: tokens are gathered per expert (transposed, so the contraction
dim H lands on SBUF partitions), padded to a per-slot capacity that is
uniform across cores so one Bass program serves all 8 cores SPMD. The device
streams each expert's weights from HBM exactly once and runs the SwiGLU
matmuls in float32r (full-rate PE) with fp32 accumulation, scaling each
token row by its combine weight. Shared experts are token-sharded: core m
computes the shared MLP for tokens [512m, 512(m+1)). The host scatter-adds
per-expert outputs back to token order (the unshard step).

Matmul layout trick: h1^T = matmul(lhsT=W_gate[H,I], rhs=x^T[H,C]) gives
[I, C] directly, and y = matmul(lhsT=a^T[I,C], rhs=W_down[I,H]) gives
[C, H] — no on-device transposes anywhere.
"""

import os
import numpy as np

# ---- problem constants (nn_DeepseekMoE_42236708389026) ----
T, H = 4096, 1024
E, I = 64, 704
IP = 768                    # I padded to a multiple of 128 (zero-padded weights)
TOP_K, N_GROUP, TOPK_GROUP = 8, 8, 4
N_SHARED = 2
I2 = N_SHARED * I           # 1408 = 11 * 128
ROUTED_SCALE = 2.5
NCORES = 8
EL = E // NCORES            # 8 local experts / core
TS = T // NCORES            # 512 shared-slab tokens / core
P = 128
NT = 512                    # token tile (PSUM bank = 512 fp32)
KO = H // P                 # 8

_BUILD_CACHE: dict = {}
LAST_EXEC_NS = None
LAST_RESULTS = None


def _routing(x, gate_w, gate_bias):
    """Replicates the reference _grouped_topk bit-exactly (jax on CPU)."""
    import jax
    import jax.numpy as jnp

    cpu = jax.devices("cpu")[0]
    with jax.default_device(cpu):
        x = jnp.asarray(x)
        gate_w = jnp.asarray(gate_w)
        gate_bias = jnp.asarray(gate_bias)
        logits = jnp.einsum("th,eh->te", x, gate_w)
        scores = jax.nn.sigmoid(logits)
        sc = scores + gate_bias[None, :]
        g = sc.reshape(-1, N_GROUP, E // N_GROUP)
        group_scores = jnp.sum(jax.lax.top_k(g, 2)[0], axis=-1)
        _, group_idx = jax.lax.top_k(group_scores, TOPK_GROUP)
        group_mask = jnp.sum(jax.nn.one_hot(group_idx, N_GROUP, dtype=sc.dtype), axis=1) > 0
        masked = jnp.where(group_mask[:, :, None], g, -jnp.inf).reshape(-1, E)
        _, topk_idx = jax.lax.top_k(masked, TOP_K)
        topk_w = jnp.take_along_axis(scores, topk_idx, axis=-1)
        topk_w = topk_w / jnp.sum(topk_w, axis=-1, keepdims=True)
        topk_w = topk_w * ROUTED_SCALE
        return np.asarray(topk_w, np.float32), np.asarray(topk_idx, np.int32)


def _token_tiles(C):
    out = []
    off = 0
    while off < C:
        sz = min(NT, C - off)
        out.append((off, sz))
        off += sz
    return out


def _emit_swiglu_slot(nc, tc, pools, IT, xgt, xcol0, C, wg_s, wu_s, wd_s,
                      cw_t, out, orow0, tagp):
    """One expert slot: out[orow0:orow0+C] = swiglu(x) (optionally row-scaled).

    xgt: DRAM [H, *] token matrix (transposed); columns [xcol0, xcol0+C).
    wg_s/wu_s: DRAM [H, IT*128] ; wd_s: DRAM [IT*128, H].
    cw_t: SBUF [P, ncols] per-chunk combine weights (col = global row / 128),
          or None for the shared slot.
    out: DRAM [*, H]; rows [orow0, orow0+C).
    """
    import concourse.mybir as mybir
    F32 = mybir.dt.float32
    F32R = mybir.dt.float32r
    AF = mybir.ActivationFunctionType
    wpool, wdpool, xpool, apool, ypool, pp = pools
    IF = IT * P

    wg_t = wpool.tile([P, KO, IF], F32R, tag=f"{tagp}wg")
    wu_t = wpool.tile([P, KO, IF], F32R, tag=f"{tagp}wu")
    wd_t = wdpool.tile([P, IT, H], F32R, tag=f"{tagp}wd")
    for ko in range(KO):
        nc.sync.dma_start(wg_t[:, ko], wg_s[ko * P:(ko + 1) * P, :])
        nc.sync.dma_start(wu_t[:, ko], wu_s[ko * P:(ko + 1) * P, :])
    for it in range(IT):
        nc.sync.dma_start(wd_t[:, it], wd_s[it * P:(it + 1) * P, :])

    for (ntoff, ntsz) in _token_tiles(C):
        xg_t = xpool.tile([P, KO, NT], F32R, tag=f"{tagp}xg", name="xg_t")[:, :, :ntsz]
        for ko in range(KO):
            nc.sync.dma_start(
                xg_t[:, ko],
                xgt[ko * P:(ko + 1) * P, xcol0 + ntoff: xcol0 + ntoff + ntsz])

        a_t = apool.tile([P, IT, NT], F32R, tag=f"{tagp}aT", name="a_t")[:, :, :ntsz]
        for it in range(IT):
            ps1 = pp.tile([P, NT], F32, tag="ps1", name="ps1")[:, :ntsz]
            ps2 = pp.tile([P, NT], F32, tag="ps2", name="ps2")[:, :ntsz]
            for ko in range(KO):
                nc.tensor.matmul(
                    ps1,
                    lhsT=wg_t[:, ko, it * P:(it + 1) * P],
                    rhs=xg_t[:, ko],
                    start=(ko == 0), stop=(ko == KO - 1))
                nc.tensor.matmul(
                    ps2,
                    lhsT=wu_t[:, ko, it * P:(it + 1) * P],
                    rhs=xg_t[:, ko],
                    start=(ko == 0), stop=(ko == KO - 1))
            nc.scalar.activation(a_t[:, it], ps1, AF.Sigmoid)
            nc.vector.tensor_mul(a_t[:, it], a_t[:, it], ps1)
            nc.vector.tensor_mul(a_t[:, it], a_t[:, it], ps2)

        for ch in range((ntsz + P - 1) // P):
            m = min(P, ntsz - ch * P)
            for hh in range(H // NT):
                ps3 = pp.tile([P, NT], F32, tag="ps3", name="ps3")[:m]
                for it in range(IT):
                    nc.tensor.matmul(
                        ps3,
                        lhsT=a_t[:, it, ch * P: ch * P + m],
                        rhs=wd_t[:, it, hh * NT:(hh + 1) * NT],
                        start=(it == 0), stop=(it == IT - 1))
                y_t = ypool.tile([P, NT], F32, tag=f"{tagp}y", name="y_t")[:m]
                row0 = orow0 + ntoff + ch * P
                if cw_t is not None:
                    nc.vector.tensor_scalar_mul(y_t, ps3, cw_t[:m, row0 // P, None])
                else:
                    nc.vector.tensor_copy(y_t, ps3)
                nc.sync.dma_start(out[row0: row0 + m, hh * NT:(hh + 1) * NT], y_t)


def _build(Cs, offs, CT, reps=1):
    """Build + schedule the SPMD Bass program for slot capacities Cs.

    reps>1 wraps the body in a hardware loop (benchmarking variant: the
    per-iteration slope isolates device exec from launch overhead).
    """
    import contextlib
    import concourse.mybir as mybir
    from concourse import bacc
    import concourse.tile as tile

    F32 = mybir.dt.float32
    F32R = mybir.dt.float32r

    nc = bacc.Bacc("TRN2", target_bir_lowering=False, debug=False)
    xgt = nc.dram_tensor("xgt", [H, CT], F32R, kind="ExternalInput")
    cw = nc.dram_tensor("cw", [CT], F32, kind="ExternalInput")
    wg = nc.dram_tensor("wg", [EL, H, IP], F32R, kind="ExternalInput")
    wu = nc.dram_tensor("wu", [EL, H, IP], F32R, kind="ExternalInput")
    wd = nc.dram_tensor("wd", [EL, IP, H], F32R, kind="ExternalInput")
    xst = nc.dram_tensor("xst", [H, TS], F32R, kind="ExternalInput")
    wsg = nc.dram_tensor("wsg", [H, I2], F32R, kind="ExternalInput")
    wsu = nc.dram_tensor("wsu", [H, I2], F32R, kind="ExternalInput")
    wsd = nc.dram_tensor("wsd", [I2, H], F32R, kind="ExternalInput")
    y = nc.dram_tensor("y", [CT, H], F32, kind="ExternalOutput")
    ys = nc.dram_tensor("ys", [TS, H], F32, kind="ExternalOutput")

    with tile.TileContext(nc) as tc:
      with (tc.For_i(0, reps, 1) if reps > 1 else contextlib.nullcontext()):
        with tc.tile_pool(name="psum", bufs=2, space="PSUM") as pp:
            # --- routed expert slots ---
            with (
                tc.tile_pool(name="rw", bufs=2) as wpool,
                tc.tile_pool(name="rwd", bufs=1) as wdpool,
                tc.tile_pool(name="rx", bufs=2) as xpool,
                tc.tile_pool(name="ra", bufs=2) as apool,
                tc.tile_pool(name="ry", bufs=2) as ypool,
                tc.tile_pool(name="rc", bufs=1) as cpool,
            ):
                cw_t = cpool.tile([P, CT // P], F32, tag="cw")
                nc.sync.dma_start(cw_t, cw.rearrange("(n p) -> p n", p=P))
                for s in range(EL):
                    C = int(Cs[s])
                    if C:
                        _emit_swiglu_slot(
                            nc, tc, (wpool, wdpool, xpool, apool, ypool, pp),
                            IP // P, xgt, int(offs[s]), C,
                            wg[s], wu[s], wd[s], cw_t, y, int(offs[s]), "r")
            # --- shared experts (token shard) ---
            with (
                tc.tile_pool(name="sw", bufs=1) as swpool,
                tc.tile_pool(name="sx", bufs=1) as sxpool,
                tc.tile_pool(name="sa", bufs=1) as sapool,
                tc.tile_pool(name="sy", bufs=2) as sypool,
            ):
                _emit_swiglu_slot(
                    nc, tc, (swpool, swpool, sxpool, sapool, sypool, pp),
                    I2 // P, xst, 0, TS,
                    wsg, wsu, wsd, None, ys, 0, "s")

    nc.compile()
    return nc


def _prepare(hidden_states, gate_w, gate_bias, w_gate, w_up, w_down,
             ws_gate, ws_up, ws_down):
    """Host routing + dispatch. Returns (nc, in_maps, meta) for the SPMD run."""
    hs = np.ascontiguousarray(np.asarray(hidden_states, np.float32))
    gate_w = np.asarray(gate_w, np.float32)
    gate_bias = np.asarray(gate_bias, np.float32)
    w_gate = np.asarray(w_gate, np.float32)
    w_up = np.asarray(w_up, np.float32)
    w_down = np.asarray(w_down, np.float32)
    ws_gate = np.ascontiguousarray(np.asarray(ws_gate, np.float32))
    ws_up = np.ascontiguousarray(np.asarray(ws_up, np.float32))
    ws_down = np.ascontiguousarray(np.asarray(ws_down, np.float32))

    # ---- host: router + dispatch (the sharding step) ----
    topk_w, topk_idx = _routing(hs, gate_w, gate_bias)

    rows_of = []
    wts_of = []
    counts = np.zeros(E, np.int64)
    for e in range(E):
        rr, kk = np.nonzero(topk_idx == e)
        rows_of.append(rr)
        wts_of.append(topk_w[rr, kk])
        counts[e] = len(rr)

    # expert -> (core, slot) assignment: we own the sharding, so sort experts
    # by token count desc and give slot s of core m the (8s+m)-th largest.
    # Sorted tiering minimizes sum-of-tier-maxima = padded compute per core.
    order = np.argsort(-counts, kind="stable")
    perm = order.reshape(EL, NCORES).T              # [NCORES, EL]
    slot_counts = counts[perm]                      # [NCORES, EL]
    # capacity = tier max rounded up to even (fp32r ISA: innermost moving /
    # psum-dst counts must be even — fp32r is a bf16-pair decomposition)
    Cs = ((slot_counts.max(axis=0) + 1) // 2 * 2).astype(np.int64)
    # buffer offsets 128-aligned (cw chunk addressing); capacities exact
    offs = np.concatenate([[0], np.cumsum((Cs + P - 1) // P * P)[:-1]])
    CT = int(((Cs[-1] + P - 1) // P * P) + offs[-1])

    hsT = np.ascontiguousarray(hs.T)                # [H, T]

    in_maps = []
    for m in range(NCORES):
        xgt_m = np.zeros((H, CT), np.float32)
        cw_m = np.zeros(CT, np.float32)
        wg_m = np.zeros((EL, H, IP), np.float32)
        wu_m = np.zeros((EL, H, IP), np.float32)
        wd_m = np.zeros((EL, IP, H), np.float32)
        for s in range(EL):
            e = perm[m, s]
            n = counts[e]
            o = offs[s]
            xgt_m[:, o:o + n] = hsT[:, rows_of[e]]
            cw_m[o:o + n] = wts_of[e]
            wg_m[s, :, :I] = w_gate[e]
            wu_m[s, :, :I] = w_up[e]
            wd_m[s, :I, :] = w_down[e]
        in_maps.append(dict(
            xgt=xgt_m, cw=cw_m, wg=wg_m, wu=wu_m, wd=wd_m,
            xst=np.ascontiguousarray(hsT[:, m * TS:(m + 1) * TS]),
            wsg=ws_gate, wsu=ws_up, wsd=ws_down))

    key = tuple(int(c) for c in Cs)
    if key not in _BUILD_CACHE:
        _BUILD_CACHE[key] = _build(Cs, offs, CT)
    nc = _BUILD_CACHE[key]

    meta = dict(perm=perm, counts=counts, offs=offs, rows_of=rows_of,
                Cs=Cs, CT=CT, key=key)
    return nc, in_maps, meta


def _combine(results, meta):
    """Host unshard: scatter-add per-expert outputs back to token order."""
    perm, counts, offs, rows_of = (
        meta["perm"], meta["counts"], meta["offs"], meta["rows_of"])
    out = np.zeros((T, H), np.float32)
    for m in range(NCORES):
        y_m = results[m]["y"]
        for s in range(EL):
            e = perm[m, s]
            n = counts[e]
            o = offs[s]
            out[rows_of[e]] += y_m[o:o + n]
        out[m * TS:(m + 1) * TS] += results[m]["ys"]
    return out


def kernel(hidden_states, gate_w, gate_bias, w_gate, w_up, w_down,
           ws_gate, ws_up, ws_down):
    from concourse import bass_utils

    nc, in_maps, meta = _prepare(
        hidden_states, gate_w, gate_bias, w_gate, w_up, w_down,
        ws_gate, ws_up, ws_down)
    res = bass_utils.run_bass_kernel_spmd(
        nc, in_maps, core_ids=list(range(NCORES)))
    return _combine(res.results, meta)


# revision 19
# speedup vs baseline: 10.0361x; 5.3670x over previous
"""DeepSeek-MoE Trainium2 kernel (8-core expert-parallel).

Strategy (per spec sharding_hint): expert-parallel. Each of the 8 cores owns
8 of the 64 routed experts. The host computes the router (grouped top-k) and
dispatches: tokens are gathered per expert (transposed, so the contraction
dim H lands on SBUF partitions), padded to a per-slot capacity that is
uniform across cores so one Bass program serves all 8 cores SPMD. The device
streams each expert's weights from HBM exactly once and runs the SwiGLU
matmuls in float32r (full-rate PE) with fp32 accumulation, scaling each
token row by its combine weight. Shared experts are token-sharded: core m
computes the shared MLP for tokens [512m, 512(m+1)). The host scatter-adds
per-expert outputs back to token order (the unshard step).

Matmul layout trick: h1^T = matmul(lhsT=W_gate[H,I], rhs=x^T[H,C]) gives
[I, C] directly, and y = matmul(lhsT=a^T[I,C], rhs=W_down[I,H]) gives
[C, H] — no on-device transposes anywhere.
"""

import os
import numpy as np

# ---- problem constants (nn_DeepseekMoE_42236708389026) ----
T, H = 4096, 1024
E, I = 64, 704
IP = 768                    # I padded to a multiple of 128 (zero-padded weights)
TOP_K, N_GROUP, TOPK_GROUP = 8, 8, 4
N_SHARED = 2
I2 = N_SHARED * I           # 1408 = 11 * 128
ROUTED_SCALE = 2.5
NCORES = 8
EL = E // NCORES            # 8 local experts / core
TS = T // NCORES            # 512 shared-slab tokens / core
P = 128
NT = 512                    # token tile (PSUM bank = 512 fp32)
KO = H // P                 # 8

_BUILD_CACHE: dict = {}
LAST_EXEC_NS = None
LAST_RESULTS = None


def _routing(x, gate_w, gate_bias):
    """Replicates the reference _grouped_topk bit-exactly (jax on CPU)."""
    import jax
    import jax.numpy as jnp

    cpu = jax.devices("cpu")[0]
    with jax.default_device(cpu):
        x = jnp.asarray(x)
        gate_w = jnp.asarray(gate_w)
        gate_bias = jnp.asarray(gate_bias)
        logits = jnp.einsum("th,eh->te", x, gate_w)
        scores = jax.nn.sigmoid(logits)
        sc = scores + gate_bias[None, :]
        g = sc.reshape(-1, N_GROUP, E // N_GROUP)
        group_scores = jnp.sum(jax.lax.top_k(g, 2)[0], axis=-1)
        _, group_idx = jax.lax.top_k(group_scores, TOPK_GROUP)
        group_mask = jnp.sum(jax.nn.one_hot(group_idx, N_GROUP, dtype=sc.dtype), axis=1) > 0
        masked = jnp.where(group_mask[:, :, None], g, -jnp.inf).reshape(-1, E)
        _, topk_idx = jax.lax.top_k(masked, TOP_K)
        topk_w = jnp.take_along_axis(scores, topk_idx, axis=-1)
        topk_w = topk_w / jnp.sum(topk_w, axis=-1, keepdims=True)
        topk_w = topk_w * ROUTED_SCALE
        return np.asarray(topk_w, np.float32), np.asarray(topk_idx, np.int32)


def _token_tiles(C):
    out = []
    off = 0
    while off < C:
        sz = min(NT, C - off)
        out.append((off, sz))
        off += sz
    return out


def _emit_swiglu_slot(nc, tc, pools, IW, ITD, xgt, xcol0, C, wg_s, wu_s, wd_s,
                      cw_t, out, orow0, tagp):
    """One expert slot: out[orow0:orow0+C] = swiglu(x) (optionally row-scaled).

    xgt: DRAM [H, *] token matrix (transposed); columns [xcol0, xcol0+C).
    wg_s/wu_s: DRAM [H, IW] (unpadded); wd_s: DRAM [ITD*128, H] where rows
    [IW, ITD*128) are zero — they null out the garbage a_t rows of the last
    partial I-tile.
    cw_t: SBUF [P, ncols] per-chunk combine weights (col = global row / 128),
          or None for the shared slot.
    out: DRAM [*, H]; rows [orow0, orow0+C).
    """
    import concourse.mybir as mybir
    F32 = mybir.dt.float32
    F32R = mybir.dt.float32r
    AF = mybir.ActivationFunctionType
    wpool, wdpool, xpool, apool, ypool, pp = pools

    wg_t = wpool.tile([P, KO, IW], F32R, tag=f"{tagp}wg")
    wu_t = wpool.tile([P, KO, IW], F32R, tag=f"{tagp}wu")
    wd_t = wdpool.tile([P, ITD, H], F32R, tag=f"{tagp}wd")

    def load_x(ntoff, ntsz):
        xg_t = xpool.tile([P, KO, NT], F32R, tag=f"{tagp}xg", name="xg_t")[:, :, :ntsz]
        for ko in range(KO):
            nc.sync.dma_start(
                xg_t[:, ko],
                xgt[ko * P:(ko + 1) * P, xcol0 + ntoff: xcol0 + ntoff + ntsz])
        return xg_t

    tiles = _token_tiles(C)
    # first nt's tokens before the weights: the first matmul needs both, and
    # x is the smaller load; later slots hide all of this behind prefetch
    xg_first = load_x(*tiles[0])
    for ko in range(KO):
        nc.sync.dma_start(wg_t[:, ko], wg_s[ko * P:(ko + 1) * P, :])
        nc.sync.dma_start(wu_t[:, ko], wu_s[ko * P:(ko + 1) * P, :])
    for it in range(ITD):
        nc.sync.dma_start(wd_t[:, it], wd_s[it * P:(it + 1) * P, :])

    for nti, (ntoff, ntsz) in enumerate(tiles):
        xg_t = xg_first if nti == 0 else load_x(ntoff, ntsz)

        a_t = apool.tile([P, ITD, NT], F32R, tag=f"{tagp}aT", name="a_t")[:, :, :ntsz]
        for it in range(ITD):
            mi = min(P, IW - it * P)        # I-rows in this tile (last may be 64)
            ps1 = pp.tile([P, NT], F32, tag="ps1", name="ps1", bufs=3)[:mi, :ntsz]
            ps2 = pp.tile([P, NT], F32, tag="ps2", name="ps2", bufs=3)[:mi, :ntsz]
            for ko in range(KO):
                nc.tensor.matmul(
                    ps1,
                    lhsT=wg_t[:, ko, it * P: it * P + mi],
                    rhs=xg_t[:, ko],
                    start=(ko == 0), stop=(ko == KO - 1))
                nc.tensor.matmul(
                    ps2,
                    lhsT=wu_t[:, ko, it * P: it * P + mi],
                    rhs=xg_t[:, ko],
                    start=(ko == 0), stop=(ko == KO - 1))
            if mi < P:
                # zero the tail rows: wd's zero rows null them in mm3, but
                # uninitialized SBUF could hold NaN/Inf and NaN*0=NaN
                nc.vector.memset(a_t[mi:, it].bitcast(F32), 0.0)
            nc.scalar.activation(a_t[:mi, it], ps1, AF.Sigmoid)
            nc.vector.tensor_mul(a_t[:mi, it], a_t[:mi, it], ps1)
            nc.vector.tensor_mul(a_t[:mi, it], a_t[:mi, it], ps2)

        for ch in range((ntsz + P - 1) // P):
            m = min(P, ntsz - ch * P)
            for hh in range(H // NT):
                ps3 = pp.tile([P, NT], F32, tag="ps3", name="ps3")[:m]
                for it in range(ITD):
                    nc.tensor.matmul(
                        ps3,
                        lhsT=a_t[:, it, ch * P: ch * P + m],
                        rhs=wd_t[:, it, hh * NT:(hh + 1) * NT],
                        start=(it == 0), stop=(it == ITD - 1))
                y_t = ypool.tile([P, NT], F32, tag=f"{tagp}y", name="y_t")[:m]
                row0 = orow0 + ntoff + ch * P
                if cw_t is not None:
                    nc.vector.tensor_scalar_mul(y_t, ps3, cw_t[:m, row0 // P, None])
                else:
                    nc.vector.tensor_copy(y_t, ps3)
                nc.sync.dma_start(out[row0: row0 + m, hh * NT:(hh + 1) * NT], y_t)


def _build(Cs, offs, CT, reps=1):
    """Build + schedule the SPMD Bass program.

    10 uniform slots: 8 routed experts plus the shared expert split into two
    I=704 column-halves (identical tile shapes -> one fully pipelined loop,
    no phase boundary). reps>1 wraps the body in a hardware loop (bench
    variant: per-iteration slope isolates device exec from launch overhead).
    """
    import contextlib
    import concourse.mybir as mybir
    from concourse import bacc
    import concourse.tile as tile

    F32 = mybir.dt.float32
    F32R = mybir.dt.float32r

    NSLOT = EL + 2
    CTX = CT + TS               # xgt cols: routed capacity + shared slab
    CTY = CT + 2 * TS           # y rows: two shared halves write separately

    nc = bacc.Bacc("TRN2", target_bir_lowering=False, debug=False)
    xgt = nc.dram_tensor("xgt", [H, CTX], F32R, kind="ExternalInput")
    cw = nc.dram_tensor("cw", [CTY], F32, kind="ExternalInput")
    wg = nc.dram_tensor("wg", [NSLOT, H, I], F32R, kind="ExternalInput")
    wu = nc.dram_tensor("wu", [NSLOT, H, I], F32R, kind="ExternalInput")
    wd = nc.dram_tensor("wd", [NSLOT, IP, H], F32R, kind="ExternalInput")
    y = nc.dram_tensor("y", [CTY, H], F32, kind="ExternalOutput")

    slot_geom = [(int(offs[s]), int(offs[s]), int(Cs[s])) for s in range(EL)]
    slot_geom += [(CT, CT, TS), (CT, CT + TS, TS)]   # (xcol0, orow0, C)

    with tile.TileContext(nc) as tc:
      with (tc.For_i(0, reps, 1) if reps > 1 else contextlib.nullcontext()):
        with (
            tc.tile_pool(name="psum", bufs=2, space="PSUM") as pp,
            tc.tile_pool(name="rw", bufs=2) as wpool,
            tc.tile_pool(name="rwd", bufs=2) as wdpool,
            tc.tile_pool(name="rx", bufs=2) as xpool,
            tc.tile_pool(name="ra", bufs=1) as apool,
            tc.tile_pool(name="ry", bufs=2) as ypool,
            tc.tile_pool(name="rc", bufs=1) as cpool,
        ):
            cw_t = cpool.tile([P, CTY // P], F32, tag="cw")
            nc.sync.dma_start(cw_t, cw.rearrange("(n p) -> p n", p=P))
            for s, (xcol0, orow0, C) in enumerate(slot_geom):
                if C:
                    _emit_swiglu_slot(
                        nc, tc, (wpool, wdpool, xpool, apool, ypool, pp),
                        I, IP // P, xgt, xcol0, C,
                        wg[s], wu[s], wd[s], cw_t, y, orow0, "r")

    nc.compile()
    return nc


def _prepare(hidden_states, gate_w, gate_bias, w_gate, w_up, w_down,
             ws_gate, ws_up, ws_down):
    """Host routing + dispatch. Returns (nc, in_maps, meta) for the SPMD run."""
    hs = np.ascontiguousarray(np.asarray(hidden_states, np.float32))
    gate_w = np.asarray(gate_w, np.float32)
    gate_bias = np.asarray(gate_bias, np.float32)
    w_gate = np.asarray(w_gate, np.float32)
    w_up = np.asarray(w_up, np.float32)
    w_down = np.asarray(w_down, np.float32)
    ws_gate = np.ascontiguousarray(np.asarray(ws_gate, np.float32))
    ws_up = np.ascontiguousarray(np.asarray(ws_up, np.float32))
    ws_down = np.ascontiguousarray(np.asarray(ws_down, np.float32))

    # ---- host: router + dispatch (the sharding step) ----
    topk_w, topk_idx = _routing(hs, gate_w, gate_bias)

    rows_of = []
    wts_of = []
    counts = np.zeros(E, np.int64)
    for e in range(E):
        rr, kk = np.nonzero(topk_idx == e)
        rows_of.append(rr)
        wts_of.append(topk_w[rr, kk])
        counts[e] = len(rr)

    # expert -> (core, slot) assignment: we own the sharding, so sort experts
    # by token count desc and give slot s of core m the (8s+m)-th largest.
    # Sorted tiering minimizes sum-of-tier-maxima = padded compute per core.
    order = np.argsort(-counts, kind="stable")
    perm = order.reshape(EL, NCORES).T              # [NCORES, EL]
    slot_counts = counts[perm]                      # [NCORES, EL]
    # capacity = tier max rounded up to even (fp32r ISA: innermost moving /
    # psum-dst counts must be even — fp32r is a bf16-pair decomposition)
    Cs = ((slot_counts.max(axis=0) + 1) // 2 * 2).astype(np.int64)
    # buffer offsets 128-aligned (cw chunk addressing); capacities exact
    offs = np.concatenate([[0], np.cumsum((Cs + P - 1) // P * P)[:-1]])
    CT = int(((Cs[-1] + P - 1) // P * P) + offs[-1])

    hsT = np.ascontiguousarray(hs.T)                # [H, T]

    NSLOT = EL + 2
    CTX = CT + TS
    CTY = CT + 2 * TS
    in_maps = []
    for m in range(NCORES):
        xgt_m = np.zeros((H, CTX), np.float32)
        cw_m = np.zeros(CTY, np.float32)
        wg_m = np.zeros((NSLOT, H, I), np.float32)
        wu_m = np.zeros((NSLOT, H, I), np.float32)
        wd_m = np.zeros((NSLOT, IP, H), np.float32)
        for s in range(EL):
            e = perm[m, s]
            n = counts[e]
            o = offs[s]
            xgt_m[:, o:o + n] = hsT[:, rows_of[e]]
            cw_m[o:o + n] = wts_of[e]
            wg_m[s] = w_gate[e]
            wu_m[s] = w_up[e]
            wd_m[s, :I, :] = w_down[e]
        # shared expert: token slab as extra x columns; its 2I intermediate
        # split into two I-halves as slots 8/9 (combine weight 1.0)
        xgt_m[:, CT:CTX] = hsT[:, m * TS:(m + 1) * TS]
        cw_m[CT:CTY] = 1.0
        for h in range(2):
            wg_m[EL + h] = ws_gate[:, h * I:(h + 1) * I]
            wu_m[EL + h] = ws_up[:, h * I:(h + 1) * I]
            wd_m[EL + h, :I, :] = ws_down[h * I:(h + 1) * I, :]
        in_maps.append(dict(xgt=xgt_m, cw=cw_m, wg=wg_m, wu=wu_m, wd=wd_m))

    key = tuple(int(c) for c in Cs)
    if key not in _BUILD_CACHE:
        _BUILD_CACHE[key] = _build(Cs, offs, CT)
    nc = _BUILD_CACHE[key]

    meta = dict(perm=perm, counts=counts, offs=offs, rows_of=rows_of,
                Cs=Cs, CT=CT, key=key)
    return nc, in_maps, meta


def _combine(results, meta):
    """Host unshard: scatter-add per-expert outputs back to token order."""
    perm, counts, offs, rows_of = (
        meta["perm"], meta["counts"], meta["offs"], meta["rows_of"])
    CT = meta["CT"]
    out = np.zeros((T, H), np.float32)
    for m in range(NCORES):
        y_m = results[m]["y"]
        for s in range(EL):
            e = perm[m, s]
            n = counts[e]
            o = offs[s]
            out[rows_of[e]] += y_m[o:o + n]
        out[m * TS:(m + 1) * TS] += y_m[CT:CT + TS] + y_m[CT + TS:CT + 2 * TS]
    return out


def kernel(hidden_states, gate_w, gate_bias, w_gate, w_up, w_down,
           ws_gate, ws_up, ws_down):
    from concourse import bass_utils

    nc, in_maps, meta = _prepare(
        hidden_states, gate_w, gate_bias, w_gate, w_up, w_down,
        ws_gate, ws_up, ws_down)
    res = bass_utils.run_bass_kernel_spmd(
        nc, in_maps, core_ids=list(range(NCORES)))
    return _combine(res.results, meta)


# revision 23
# speedup vs baseline: 11.2462x; 1.1206x over previous
"""DeepSeek-MoE Trainium2 kernel (8-core expert-parallel).

Strategy (per spec sharding_hint): expert-parallel. Each of the 8 cores owns
8 of the 64 routed experts. The host computes the router (grouped top-k) and
dispatches: tokens are gathered per expert (transposed, so the contraction
dim H lands on SBUF partitions), padded to a per-slot capacity that is
uniform across cores so one Bass program serves all 8 cores SPMD. The device
streams each expert's weights from HBM exactly once and runs the SwiGLU
matmuls in float32r (full-rate PE) with fp32 accumulation, scaling each
token row by its combine weight. Shared experts are token-sharded: core m
computes the shared MLP for tokens [512m, 512(m+1)). The host scatter-adds
per-expert outputs back to token order (the unshard step).

Matmul layout trick: h1^T = matmul(lhsT=W_gate[H,I], rhs=x^T[H,C]) gives
[I, C] directly, and y = matmul(lhsT=a^T[I,C], rhs=W_down[I,H]) gives
[C, H] — no on-device transposes anywhere.
"""

import os
import numpy as np

# ---- problem constants (nn_DeepseekMoE_42236708389026) ----
T, H = 4096, 1024
E, I = 64, 704
IP = 768                    # I padded to a multiple of 128 (zero-padded weights)
TOP_K, N_GROUP, TOPK_GROUP = 8, 8, 4
N_SHARED = 2
I2 = N_SHARED * I           # 1408 = 11 * 128
ROUTED_SCALE = 2.5
NCORES = 8
EL = E // NCORES            # 8 local experts / core
TS = T // NCORES            # 512 shared-slab tokens / core
P = 128
NT = 512                    # token tile (PSUM bank = 512 fp32)
KO = H // P                 # 8

_BUILD_CACHE: dict = {}
LAST_EXEC_NS = None
LAST_RESULTS = None


def _routing(x, gate_w, gate_bias):
    """Replicates the reference _grouped_topk bit-exactly (jax on CPU)."""
    import jax
    import jax.numpy as jnp

    cpu = jax.devices("cpu")[0]
    with jax.default_device(cpu):
        x = jnp.asarray(x)
        gate_w = jnp.asarray(gate_w)
        gate_bias = jnp.asarray(gate_bias)
        logits = jnp.einsum("th,eh->te", x, gate_w)
        scores = jax.nn.sigmoid(logits)
        sc = scores + gate_bias[None, :]
        g = sc.reshape(-1, N_GROUP, E // N_GROUP)
        group_scores = jnp.sum(jax.lax.top_k(g, 2)[0], axis=-1)
        _, group_idx = jax.lax.top_k(group_scores, TOPK_GROUP)
        group_mask = jnp.sum(jax.nn.one_hot(group_idx, N_GROUP, dtype=sc.dtype), axis=1) > 0
        masked = jnp.where(group_mask[:, :, None], g, -jnp.inf).reshape(-1, E)
        _, topk_idx = jax.lax.top_k(masked, TOP_K)
        topk_w = jnp.take_along_axis(scores, topk_idx, axis=-1)
        topk_w = topk_w / jnp.sum(topk_w, axis=-1, keepdims=True)
        topk_w = topk_w * ROUTED_SCALE
        return np.asarray(topk_w, np.float32), np.asarray(topk_idx, np.int32)


def _token_tiles(C):
    out = []
    off = 0
    while off < C:
        sz = min(NT, C - off)
        out.append((off, sz))
        off += sz
    return out


def _emit_swiglu_slot(nc, tc, pools, IW, ITD, xgt, xcol0, C, wg_s, wu_s, wd_s,
                      cw_t, out, orow0, tagp):
    """One expert slot: out[orow0:orow0+C] = swiglu(x) (optionally row-scaled).

    xgt: DRAM [H, *] token matrix (transposed); columns [xcol0, xcol0+C).
    wg_s/wu_s: DRAM [H, IW] (unpadded); wd_s: DRAM [ITD*128, H] where rows
    [IW, ITD*128) are zero — they null out the garbage a_t rows of the last
    partial I-tile.
    cw_t: SBUF [P, ncols] per-chunk combine weights (col = global row / 128),
          or None for the shared slot.
    out: DRAM [*, H]; rows [orow0, orow0+C).
    """
    import concourse.mybir as mybir
    F32 = mybir.dt.float32
    F32R = mybir.dt.float32r
    BF16 = mybir.dt.bfloat16
    AF = mybir.ActivationFunctionType
    wpool, wdpool, xpool, apool, ypool, pp = pools

    wg_t = wpool.tile([P, KO, IW], F32R, tag=f"{tagp}wg")
    wu_t = wpool.tile([P, KO, IW], F32R, tag=f"{tagp}wu")
    wd_t = wdpool.tile([P, ITD, H], F32R, tag=f"{tagp}wd")

    def load_x(ntoff, ntsz, split=False):
        xg_t = xpool.tile([P, KO, NT], F32R, tag=f"{tagp}xg", name="xg_t")[:, :, :ntsz]
        cols = xgt[:, xcol0 + ntoff: xcol0 + ntoff + ntsz]
        if split:
            # per-ko slices so the first accumulation chain starts on the
            # first ~256KB instead of the whole 2MB (matters on slot 0)
            for ko in range(KO):
                nc.sync.dma_start(xg_t[:, ko], cols[ko * P:(ko + 1) * P, :])
        else:
            nc.sync.dma_start(xg_t, cols.rearrange("(ko p) n -> p ko n", p=P))
        return xg_t

    tiles = _token_tiles(C)
    # first nt's tokens before the weights: the first matmul needs both, and
    # x is the smaller load; later slots hide all of this behind prefetch
    xg_first = load_x(*tiles[0], split=True)
    nc.sync.dma_start(wg_t, wg_s.rearrange("(ko p) i -> p ko i", p=P))
    nc.sync.dma_start(wu_t, wu_s.rearrange("(ko p) i -> p ko i", p=P))
    nc.sync.dma_start(wd_t, wd_s.rearrange("(it p) h -> p it h", p=P))

    for nti, (ntoff, ntsz) in enumerate(tiles):
        xg_t = xg_first if nti == 0 else load_x(ntoff, ntsz)

        a_t = apool.tile([P, ITD, NT], F32R, tag=f"{tagp}aT", name="a_t")[:, :, :ntsz]
        for it in range(ITD):
            mi = min(P, IW - it * P)        # I-rows in this tile (last may be 64)
            ps1 = pp.tile([P, NT], F32, tag="ps1", name="ps1", bufs=3)[:mi, :ntsz]
            ps2 = pp.tile([P, NT], F32, tag="ps2", name="ps2", bufs=3)[:mi, :ntsz]
            for ko in range(KO):
                nc.tensor.matmul(
                    ps1,
                    lhsT=wg_t[:, ko, it * P: it * P + mi],
                    rhs=xg_t[:, ko],
                    start=(ko == 0), stop=(ko == KO - 1))
                nc.tensor.matmul(
                    ps2,
                    lhsT=wu_t[:, ko, it * P: it * P + mi],
                    rhs=xg_t[:, ko],
                    start=(ko == 0), stop=(ko == KO - 1))
            if mi < P:
                # zero the tail rows: wd's zero rows null them in mm3, but
                # uninitialized SBUF could hold NaN/Inf and NaN*0=NaN
                nc.vector.memset(a_t[mi:, it].bitcast(F32), 0.0)
            nc.scalar.activation(a_t[:mi, it], ps1, AF.Sigmoid)
            nc.vector.tensor_mul(a_t[:mi, it], a_t[:mi, it], ps1)
            nc.vector.tensor_mul(a_t[:mi, it], a_t[:mi, it], ps2)

        for ch in range((ntsz + P - 1) // P):
            m = min(P, ntsz - ch * P)
            row0 = orow0 + ntoff + ch * P
            y_t = ypool.tile([P, H], BF16, tag=f"{tagp}y", name="y_t")[:m]
            for hh in range(H // NT):
                ps3 = pp.tile([P, NT], F32, tag="ps3", name="ps3")[:m]
                for it in range(ITD):
                    nc.tensor.matmul(
                        ps3,
                        lhsT=a_t[:, it, ch * P: ch * P + m],
                        rhs=wd_t[:, it, hh * NT:(hh + 1) * NT],
                        start=(it == 0), stop=(it == ITD - 1))
                nc.vector.tensor_scalar_mul(
                    y_t[:, hh * NT:(hh + 1) * NT], ps3, cw_t[:m, row0 // P, None])
            nc.sync.dma_start(out[row0: row0 + m, :], y_t)


def _build(Cs, offs, CT, reps=1):
    """Build + schedule the SPMD Bass program.

    10 uniform slots: 8 routed experts plus the shared expert split into two
    I=704 column-halves (identical tile shapes -> one fully pipelined loop,
    no phase boundary). reps>1 wraps the body in a hardware loop (bench
    variant: per-iteration slope isolates device exec from launch overhead).
    """
    import contextlib
    import concourse.mybir as mybir
    from concourse import bacc
    import concourse.tile as tile

    F32 = mybir.dt.float32
    F32R = mybir.dt.float32r
    BF16 = mybir.dt.bfloat16

    NSLOT = EL + 2
    CTX = CT + TS               # xgt cols: routed capacity + shared slab
    CTY = CT + 2 * TS           # y rows: two shared halves write separately

    nc = bacc.Bacc("TRN2", target_bir_lowering=False, debug=False)
    xgt = nc.dram_tensor("xgt", [H, CTX], F32R, kind="ExternalInput")
    cw = nc.dram_tensor("cw", [CTY], F32, kind="ExternalInput")
    wg = nc.dram_tensor("wg", [NSLOT, H, I], F32R, kind="ExternalInput")
    wu = nc.dram_tensor("wu", [NSLOT, H, I], F32R, kind="ExternalInput")
    wd = nc.dram_tensor("wd", [NSLOT, IP, H], F32R, kind="ExternalInput")
    y = nc.dram_tensor("y", [CTY, H], BF16, kind="ExternalOutput")

    slot_geom = [(int(offs[s]), int(offs[s]), int(Cs[s])) for s in range(EL)]
    slot_geom += [(CT, CT, TS), (CT, CT + TS, TS)]   # (xcol0, orow0, C)

    with tile.TileContext(nc) as tc:
      with (tc.For_i(0, reps, 1) if reps > 1 else contextlib.nullcontext()):
        with (
            tc.tile_pool(name="psum", bufs=2, space="PSUM") as pp,
            tc.tile_pool(name="rw", bufs=2) as wpool,
            tc.tile_pool(name="rwd", bufs=2) as wdpool,
            tc.tile_pool(name="rx", bufs=2) as xpool,
            tc.tile_pool(name="ra", bufs=2) as apool,
            tc.tile_pool(name="ry", bufs=2) as ypool,
            tc.tile_pool(name="rc", bufs=1) as cpool,
        ):
            cw_t = cpool.tile([P, CTY // P], F32, tag="cw")
            nc.sync.dma_start(cw_t, cw.rearrange("(n p) -> p n", p=P))
            for s, (xcol0, orow0, C) in enumerate(slot_geom):
                if C:
                    _emit_swiglu_slot(
                        nc, tc, (wpool, wdpool, xpool, apool, ypool, pp),
                        I, IP // P, xgt, xcol0, C,
                        wg[s], wu[s], wd[s], cw_t, y, orow0, "r")

    nc.compile()
    return nc


def _prepare(hidden_states, gate_w, gate_bias, w_gate, w_up, w_down,
             ws_gate, ws_up, ws_down):
    """Host routing + dispatch. Returns (nc, in_maps, meta) for the SPMD run."""
    hs = np.ascontiguousarray(np.asarray(hidden_states, np.float32))
    gate_w = np.asarray(gate_w, np.float32)
    gate_bias = np.asarray(gate_bias, np.float32)
    w_gate = np.asarray(w_gate, np.float32)
    w_up = np.asarray(w_up, np.float32)
    w_down = np.asarray(w_down, np.float32)
    ws_gate = np.ascontiguousarray(np.asarray(ws_gate, np.float32))
    ws_up = np.ascontiguousarray(np.asarray(ws_up, np.float32))
    ws_down = np.ascontiguousarray(np.asarray(ws_down, np.float32))

    # ---- host: router + dispatch (the sharding step) ----
    topk_w, topk_idx = _routing(hs, gate_w, gate_bias)

    rows_of = []
    wts_of = []
    counts = np.zeros(E, np.int64)
    for e in range(E):
        rr, kk = np.nonzero(topk_idx == e)
        rows_of.append(rr)
        wts_of.append(topk_w[rr, kk])
        counts[e] = len(rr)

    # expert -> (core, slot) assignment: we own the sharding, so sort experts
    # by token count desc and give slot s of core m the (8s+m)-th largest.
    # Sorted tiering minimizes sum-of-tier-maxima = padded compute per core.
    order = np.argsort(-counts, kind="stable")
    perm = order.reshape(EL, NCORES).T              # [NCORES, EL]
    slot_counts = counts[perm]                      # [NCORES, EL]
    # capacity = tier max rounded up to even (fp32r ISA: innermost moving /
    # psum-dst counts must be even — fp32r is a bf16-pair decomposition)
    Cs = ((slot_counts.max(axis=0) + 1) // 2 * 2).astype(np.int64)
    # buffer offsets 128-aligned (cw chunk addressing); capacities exact
    offs = np.concatenate([[0], np.cumsum((Cs + P - 1) // P * P)[:-1]])
    CT = int(((Cs[-1] + P - 1) // P * P) + offs[-1])

    hsT = np.ascontiguousarray(hs.T)                # [H, T]

    NSLOT = EL + 2
    CTX = CT + TS
    CTY = CT + 2 * TS
    in_maps = []
    for m in range(NCORES):
        xgt_m = np.zeros((H, CTX), np.float32)
        cw_m = np.zeros(CTY, np.float32)
        wg_m = np.zeros((NSLOT, H, I), np.float32)
        wu_m = np.zeros((NSLOT, H, I), np.float32)
        wd_m = np.zeros((NSLOT, IP, H), np.float32)
        for s in range(EL):
            e = perm[m, s]
            n = counts[e]
            o = offs[s]
            xgt_m[:, o:o + n] = hsT[:, rows_of[e]]
            cw_m[o:o + n] = wts_of[e]
            wg_m[s] = w_gate[e]
            wu_m[s] = w_up[e]
            wd_m[s, :I, :] = w_down[e]
        # shared expert: token slab as extra x columns; its 2I intermediate
        # split into two I-halves as slots 8/9 (combine weight 1.0)
        xgt_m[:, CT:CTX] = hsT[:, m * TS:(m + 1) * TS]
        cw_m[CT:CTY] = 1.0
        for h in range(2):
            wg_m[EL + h] = ws_gate[:, h * I:(h + 1) * I]
            wu_m[EL + h] = ws_up[:, h * I:(h + 1) * I]
            wd_m[EL + h, :I, :] = ws_down[h * I:(h + 1) * I, :]
        in_maps.append(dict(xgt=xgt_m, cw=cw_m, wg=wg_m, wu=wu_m, wd=wd_m))

    key = tuple(int(c) for c in Cs)
    if key not in _BUILD_CACHE:
        _BUILD_CACHE[key] = _build(Cs, offs, CT)
    nc = _BUILD_CACHE[key]

    meta = dict(perm=perm, counts=counts, offs=offs, rows_of=rows_of,
                Cs=Cs, CT=CT, key=key)
    return nc, in_maps, meta


def _combine(results, meta):
    """Host unshard: scatter-add per-expert outputs back to token order."""
    perm, counts, offs, rows_of = (
        meta["perm"], meta["counts"], meta["offs"], meta["rows_of"])
    CT = meta["CT"]
    out = np.zeros((T, H), np.float32)
    for m in range(NCORES):
        y_m = np.asarray(results[m]["y"], np.float32)
        for s in range(EL):
            e = perm[m, s]
            n = counts[e]
            o = offs[s]
            out[rows_of[e]] += y_m[o:o + n]
        out[m * TS:(m + 1) * TS] += y_m[CT:CT + TS] + y_m[CT + TS:CT + 2 * TS]
    return out


def kernel(hidden_states, gate_w, gate_bias, w_gate, w_up, w_down,
           ws_gate, ws_up, ws_down):
    from concourse import bass_utils

    nc, in_maps, meta = _prepare(
        hidden_states, gate_w, gate_bias, w_gate, w_up, w_down,
        ws_gate, ws_up, ws_down)
    res = bass_utils.run_bass_kernel_spmd(
        nc, in_maps, core_ids=list(range(NCORES)))
    return _combine(res.results, meta)
